# revision 3
# baseline (speedup 1.0000x reference)
"""KNNGraph (k=16) Bass kernel for 8 NeuronCores.

Input: x (4, 8192, 64) fp32. Output: (src, dst) int32 edge arrays of the
16-NN graph per batch (self included), matching jax.lax.top_k(-d2) order.

Sharding: core c handles batch c//2, query rows (c%2)*4096 ... +4096,
against all 8192 keys of that batch (query-row sharding, keys replicated).

Per core: for each of 32 groups of 128 query rows
  PE:  16 fp32 matmuls (K=65: 64 dims + ones row folding -|key|^2/2)
       -> PSUM chunks (128, 512) of w = q.k - |k|^2/2  (rank-equiv to -d2/2)
  ACT: copy PSUM -> SBUF w row buffer (128, 8192)
  DVE: per-chunk top-8 (max) -> 128 candidates; merge to top-16 values
       (max / match_replace / max); global indices via 2x max_index on w.
Host: assembles edges; verifies per-row sufficiency (8th-of-chunk >= 16th
overall => rescan needed) and recomputes rare ambiguous rows exactly.
"""

import numpy as np

N, M, D = 4, 8192, 64
K = 16
NCORES = 8
QROWS = M // 2          # query rows per core
NGROUPS = QROWS // 128  # 32
NCHUNK = 16             # key chunks of 512
CHUNK = M // NCHUNK     # 512
KDIM = 128              # contraction rows (64 dims + 1 ones + zero pad)

_COMPILED = {}
LAST_RUN = {}


def _build_nc():
    import concourse.bacc as bacc
    import concourse.mybir as mybir
    import concourse.tile as tile

    nc = bacc.Bacc(None)
    f32 = mybir.dt.float32
    u32 = mybir.dt.uint32

    q_d = nc.declare_dram_parameter("q", [KDIM, QROWS], f32, isOutput=False)
    kv_d = nc.declare_dram_parameter("kv", [KDIM, M], f32, isOutput=False)
    idx_d = nc.declare_dram_parameter("idx", [NGROUPS, 128, K], u32, isOutput=True)
    val_d = nc.declare_dram_parameter("val", [NGROUPS, 128, K], f32, isOutput=True)
    c8_d = nc.declare_dram_parameter("c8", [NGROUPS, 128, NCHUNK * 8], f32, isOutput=True)

    with tile.TileContext(nc) as tc:
        with (
            tc.tile_pool(name="singles", bufs=1) as singles,
            tc.tile_pool(name="wbuf", bufs=2) as wpool,
            tc.tile_pool(name="psum", bufs=8, space="PSUM") as psum,
            tc.tile_pool(name="cands", bufs=2) as cands,
            tc.tile_pool(name="smalls", bufs=2) as smalls,
        ):
            q_sb = singles.tile([KDIM, QROWS], f32)
            kv_sb = singles.tile([KDIM, M], f32)
            nc.gpsimd.dma_start(out=q_sb[:], in_=q_d[:])
            nc.gpsimd.dma_start(out=kv_sb[:], in_=kv_d[:])

            for g in range(NGROUPS):
                w = wpool.tile([128, M], f32, tag="w")
                c8 = cands.tile([128, NCHUNK * 8], f32, tag="c8")
                lhsT = q_sb[:, g * 128:(g + 1) * 128]
                for c in range(NCHUNK):
                    pt = psum.tile([128, CHUNK], f32, tag="pt")
                    nc.tensor.matmul(
                        pt[:], lhsT, kv_sb[:, c * CHUNK:(c + 1) * CHUNK],
                        start=True, stop=True,
                    )
                    nc.scalar.copy(out=w[:, c * CHUNK:(c + 1) * CHUNK], in_=pt[:])
                    nc.vector.max(
                        out=c8[:, c * 8:(c + 1) * 8],
                        in_=w[:, c * CHUNK:(c + 1) * CHUNK],
                    )
                v8a = smalls.tile([128, 8], f32, tag="v8a")
                v8b = smalls.tile([128, 8], f32, tag="v8b")
                c8m = smalls.tile([128, NCHUNK * 8], f32, tag="c8m")
                i8a = smalls.tile([128, 8], u32, tag="i8a")
                i8b = smalls.tile([128, 8], u32, tag="i8b")
                nc.vector.max(out=v8a[:], in_=c8[:])
                nc.vector.match_replace(
                    out=c8m[:], in_to_replace=v8a[:], in_values=c8[:],
                    imm_value=-3.0e38,
                )
                nc.vector.max(out=v8b[:], in_=c8m[:])
                nc.vector.max_index(out=i8a[:], in_max=v8a[:], in_values=w[:])
                nc.vector.max_index(out=i8b[:], in_max=v8b[:], in_values=w[:])

                nc.sync.dma_start(out=idx_d[g, :, 0:8], in_=i8a[:])
                nc.sync.dma_start(out=idx_d[g, :, 8:16], in_=i8b[:])
                nc.sync.dma_start(out=val_d[g, :, 0:8], in_=v8a[:])
                nc.sync.dma_start(out=val_d[g, :, 8:16], in_=v8b[:])
                nc.sync.dma_start(out=c8_d[g], in_=c8[:])
    if not nc.is_finalized():
        nc.finalize()
    return nc


def _prep_inputs(x):
    """Per-core input dicts. x: (N, M, D) fp32."""
    x64 = x.astype(np.float64)
    x2 = (x64 * x64).sum(-1)          # (N, M) exact-ish
    neg_half_x2 = (-0.5 * x2).astype(np.float32)
    in_maps = []
    for c in range(NCORES):
        b, h = c // 2, c % 2
        q = np.zeros((KDIM, QROWS), np.float32)
        q[:D] = x[b, h * QROWS:(h + 1) * QROWS, :].T
        q[D] = 1.0
        kv = np.zeros((KDIM, M), np.float32)
        kv[:D] = x[b].T
        kv[D] = neg_half_x2[b]
        in_maps.append({"q": q, "kv": kv})
    return in_maps


def _host_topk_row(x64, b, r):
    """Exact fp64 top-K for one row; returns (idx, order ascending d2)."""
    d2 = ((x64[b] - x64[b, r]) ** 2).sum(-1)
    part = np.argpartition(d2, K)[:K]
    order = part[np.argsort(d2[part], kind="stable")]
    return order


def kernel(x, k):
    x = np.asarray(x, dtype=np.float32)
    k = int(k)
    assert x.shape == (N, M, D) and k == K

    from concourse.bass_utils import run_bass_kernel_spmd

    if "nc" not in _COMPILED:
        _COMPILED["nc"] = _build_nc()
    nc = _COMPILED["nc"]

    in_maps = _prep_inputs(x)
    _r = run_bass_kernel_spmd(nc, in_maps, list(range(NCORES)))
    LAST_RUN["results"] = _r
    res = _r.results

    idx = np.empty((N, M, K), np.int64)
    val = np.empty((N, M, K), np.float64)
    c8 = np.empty((N, M, NCHUNK * 8), np.float64)
    for c in range(NCORES):
        b, h = c // 2, c % 2
        sl = slice(h * QROWS, (h + 1) * QROWS)
        idx[b, sl] = res[c]["idx"].reshape(QROWS, K)
        val[b, sl] = res[c]["val"].reshape(QROWS, K)
        c8[b, sl] = res[c]["c8"].reshape(QROWS, NCHUNK * 8)

    # ---- host verification / rare-row fallback -------------------------
    x64 = x.astype(np.float64)
    t16 = val[..., K - 1]                      # 16th-largest w
    m8 = c8[..., 7::8]                         # (N, M, 16) 8th of each chunk
    suspect = (m8 >= t16[..., None]).any(-1)
    # duplicate indices or non-strictly-descending values
    sv = np.sort(idx, axis=-1)
    suspect |= (sv[..., 1:] == sv[..., :-1]).any(-1)
    suspect |= (np.diff(val, axis=-1) >= 0).any(-1)
    nbad = int(suspect.sum())
    if nbad:
        for b, r in zip(*np.nonzero(suspect)):
            idx[b, r] = _host_topk_row(x64, b, r)

    offset = (np.arange(N, dtype=np.int64) * M)[:, None, None]
    src = (idx + offset).reshape(-1).astype(np.int32)
    dst = np.repeat(np.arange(N * M, dtype=np.int32), K)
    return src, dst


if __name__ == "__main__":
    rng = np.random.default_rng(0)
    xt = rng.standard_normal((N, M, D), dtype=np.float32)
    s, d = kernel(xt, 16)
    print(s[:32], d[:32])



# revision 5
# speedup vs baseline: 1.4792x; 1.4792x over previous
"""KNNGraph (k=16) Bass kernel for 8 NeuronCores.

Input: x (4, 8192, 64) fp32. Output: (src, dst) int32 edge arrays of the
16-NN graph per batch (self included), matching jax.lax.top_k(-d2) order.

Sharding: core c handles batch c//2, query rows (c%2)*4096 ... +4096,
against all 8192 keys of that batch (query-row sharding, keys replicated).

Device (per core), for each of 32 groups of 128 query rows:
  PE : 8 matmuls (K=128 contraction, N=1024) in fp16 hi/lo split:
       rows 0-63  = hi(x) dims,   rows 64-125 = lo(x) dims 0-61,
       row 126/127 = ones * (hi/lo of -|k|^2/2).
       PSUM w = q.k - |k|^2/2 (rank-equiv to -d2/2), noise sigma ~3e-3.
  DVE: per 1024-window: MAX8 (top-8 values) + FIND_INDEX8 (their local
       positions) straight from PSUM -> 64 (value, index) candidates/row.
Host: merge candidates, exact fp64 rescore of top-24 by noisy value,
      order by (d2, idx); conservative suspect checks -> exact fallback.
"""

import numpy as np

N, M, D = 4, 8192, 64
K = 16
NCORES = 8
QROWS = M // 2           # query rows per core
NGROUPS = QROWS // 128   # 32
NWIN = 8                 # windows of 1024 keys
WIN = M // NWIN          # 1024
KDIM = 128               # contraction rows
NCAND = NWIN * 8         # 64 candidates per row
RESCORE = 24             # exact-rescored candidates per row

_COMPILED = {}
LAST_RUN = {}


def _build_nc():
    import concourse.bacc as bacc
    import concourse.mybir as mybir
    import concourse.tile as tile

    nc = bacc.Bacc(None)
    f32 = mybir.dt.float32
    f16 = mybir.dt.float16
    u32 = mybir.dt.uint32

    q_d = nc.declare_dram_parameter("q", [KDIM, QROWS], f16, isOutput=False)
    kv_d = nc.declare_dram_parameter("kv", [KDIM, M], f16, isOutput=False)
    cv_d = nc.declare_dram_parameter("cv", [NGROUPS, 128, NCAND], f32, isOutput=True)
    ci_d = nc.declare_dram_parameter("ci", [NGROUPS, 128, NCAND], u32, isOutput=True)

    with tile.TileContext(nc) as tc:
        with (
            tc.tile_pool(name="singles", bufs=1) as singles,
            tc.tile_pool(name="psum", bufs=4, space="PSUM") as psum,
            tc.tile_pool(name="cands", bufs=2) as cands,
        ):
            q_sb = singles.tile([KDIM, QROWS], f16)
            kv_sb = singles.tile([KDIM, M], f16)
            nc.gpsimd.dma_start(out=q_sb[:], in_=q_d[:])
            nc.gpsimd.dma_start(out=kv_sb[:], in_=kv_d[:])

            for g in range(NGROUPS):
                cv = cands.tile([128, NCAND], f32, tag="cv")
                ci = cands.tile([128, NCAND], u32, tag="ci")
                lhsT = q_sb[:, g * 128:(g + 1) * 128]
                for w in range(NWIN):
                    pt = psum.tile([128, WIN], f32, tag="pt")
                    for hh in range(WIN // 512):
                        j0 = w * WIN + hh * 512
                        nc.tensor.matmul(
                            pt[:, hh * 512:(hh + 1) * 512], lhsT,
                            kv_sb[:, j0:j0 + 512],
                            start=True, stop=True,
                        )
                    nc.vector.max(out=cv[:, w * 8:(w + 1) * 8], in_=pt[:])
                    nc.vector.max_index(
                        out=ci[:, w * 8:(w + 1) * 8],
                        in_max=cv[:, w * 8:(w + 1) * 8],
                        in_values=pt[:],
                    )
                nc.sync.dma_start(out=cv_d[g], in_=cv[:])
                nc.sync.dma_start(out=ci_d[g], in_=ci[:])
    if not nc.is_finalized():
        nc.finalize()
    return nc


def _prep_inputs(x):
    """Per-core input dicts. x: (N, M, D) fp32."""
    x64 = x.astype(np.float64)
    hi = x.astype(np.float16)                       # (N, M, D)
    lo = (x64 - hi.astype(np.float64)).astype(np.float16)
    nrm = -0.5 * (x64 * x64).sum(-1)                # (N, M) exact-ish
    nh = nrm.astype(np.float16)
    nl = (nrm - nh.astype(np.float64)).astype(np.float16)

    in_maps = []
    for c in range(NCORES):
        b, h = c // 2, c % 2
        sl = slice(h * QROWS, (h + 1) * QROWS)
        q = np.zeros((KDIM, QROWS), np.float16)
        q[:D] = hi[b, sl, :].T
        q[D:D + 62] = lo[b, sl, :62].T
        q[126] = 1.0
        q[127] = 1.0
        kv = np.zeros((KDIM, M), np.float16)
        kv[:D] = hi[b].T
        kv[D:D + 62] = hi[b, :, :62].T
        kv[126] = nh[b]
        kv[127] = nl[b]
        # pair rows 64..125: q holds lo(query dims 0..61), kv holds hi(key dims 0..61)
        in_maps.append({"q": q, "kv": kv})
    return in_maps


def kernel(x, k):
    x = np.asarray(x, dtype=np.float32)
    k = int(k)
    assert x.shape == (N, M, D) and k == K

    from concourse.bass_utils import run_bass_kernel_spmd

    if "nc" not in _COMPILED:
        _COMPILED["nc"] = _build_nc()
    nc = _COMPILED["nc"]

    in_maps = _prep_inputs(x)
    _r = run_bass_kernel_spmd(nc, in_maps, list(range(NCORES)))
    LAST_RUN["results"] = _r
    res = _r.results

    cv = np.empty((N, M, NCAND), np.float32)
    ci = np.empty((N, M, NCAND), np.int64)
    for c in range(NCORES):
        b, h = c // 2, c % 2
        sl = slice(h * QROWS, (h + 1) * QROWS)
        cv[b, sl] = res[c]["cv"].reshape(QROWS, NCAND)
        ci[b, sl] = res[c]["ci"].reshape(QROWS, NCAND)
    # local window index -> global key index (window w occupies cols w*8..w*8+8)
    woff = (np.arange(NWIN, dtype=np.int64) * WIN).repeat(8)  # (64,)
    ci += woff[None, None, :]

    x64 = x.astype(np.float64)

    # ---- host merge: pick top-RESCORE by noisy value, rescore exactly ----
    order = np.argsort(-cv, axis=-1, kind="stable")             # (N, M, 64)
    top = order[..., :RESCORE]                                  # (N, M, 24)
    cidx = np.take_along_axis(ci, top, axis=-1)                 # (N, M, 24)

    idx16 = np.empty((N, M, K), np.int64)
    d2_16 = np.empty((N, M, K), np.float64)
    for b in range(N):
        keys = x64[b][cidx[b]]                                  # (M, 24, 64)
        diff = keys - x64[b][:, None, :]
        d2 = np.einsum("mcd,mcd->mc", diff, diff)               # (M, 24)
        # order by (d2 asc, idx asc) to match top_k(-d2) tie-breaking
        perm = np.lexsort((cidx[b], d2), axis=-1)[:, :K]        # (M, 16)
        idx16[b] = np.take_along_axis(cidx[b], perm, axis=-1)
        d2_16[b] = np.take_along_axis(d2, perm, axis=-1)

    # ---- suspect detection --------------------------------------------
    # exact w of the 16th winner: w = (|q|^2 - d2)/2 in the device's scale
    q2 = (x64 * x64).sum(-1)                                    # (N, M)
    w16 = 0.5 * (q2[..., None] - d2_16[..., K - 1:K])           # (N, M, 1)
    MARGIN = 0.1
    # (a) some window's 8th (noisy) could still beat the 16th winner
    win8 = cv[..., 7::8].astype(np.float64)                     # (N, M, 8)
    suspect = (win8 >= w16 - MARGIN).any(-1)
    # (b) some non-rescored candidate could beat the 16th winner
    v_sorted = np.take_along_axis(cv, order, axis=-1).astype(np.float64)
    suspect |= (v_sorted[..., RESCORE] >= w16[..., 0] - MARGIN)
    # (c) duplicate winner indices (max_index collisions on equal fp32 values)
    sv = np.sort(idx16, axis=-1)
    suspect |= (sv[..., 1:] == sv[..., :-1]).any(-1)

    nbad = int(suspect.sum())
    if nbad:
        for b in range(N):
            rows = np.nonzero(suspect[b])[0]
            if rows.size == 0:
                continue
            dif = x64[b][rows][:, None, :] - x64[b][None, :, :]   # (r, M, 64)
            d2r = np.einsum("rmd,rmd->rm", dif, dif)
            part = np.argpartition(d2r, K, axis=-1)[:, : K + 8]
            pd = np.take_along_axis(d2r, part, axis=-1)
            pperm = np.lexsort((part, pd), axis=-1)[:, :K]
            idx16[b, rows] = np.take_along_axis(part, pperm, axis=-1)

    offset = (np.arange(N, dtype=np.int64) * M)[:, None, None]
    src = (idx16 + offset).reshape(-1).astype(np.int32)
    dst = np.repeat(np.arange(N * M, dtype=np.int32), K)
    return src, dst


if __name__ == "__main__":
    rng = np.random.default_rng(0)
    xt = rng.standard_normal((N, M, D), dtype=np.float32)
    s, d = kernel(xt, 16)
    print(s[:32], d[:32])


# revision 10
# speedup vs baseline: 1.8330x; 1.2392x over previous
"""KNNGraph (k=16) Bass kernel for 8 NeuronCores.

Input: x (4, 8192, 64) fp32. Output: (src, dst) int32 edge arrays of the
16-NN graph per batch (self included), matching jax.lax.top_k(-d2) order.

Sharding: core c handles batch c//2, query rows (c%2)*4096 ... +4096,
against all 8192 keys of that batch (query-row sharding, keys replicated).

Device pipeline (per core), for each of 32 groups of 128 query rows,
with a pairwise-max tournament so the DVE only scans half the matrix:
  PE  : w = q.k - |k|^2/2 per 1024-key window (fp16 hi/lo split inputs,
        K=128 contraction, 2x N=512 matmuls into one PSUM tile)
  GPS : m_w = max(w[:, 0:512], w[:, 512:1024])  -- pair p = keys (p, p+512)
  DVE : per window: MAX8 + FIND_INDEX8 on the 512-wide m_w
        -> 64 (pair-value, pair-index) candidates per row
Host: expand top-24 pairs to 48 keys, exact fp64 rescore, order by
      (d2, idx); conservative suspect checks -> exact fallback.
"""

import numpy as np

N, M, D = 4, 8192, 64
K = 16
NCORES = 8
QROWS = M // 2           # query rows per core
NGROUPS = QROWS // 128   # 32
NWIN = 8                 # windows of 1024 keys
WIN = M // NWIN          # 1024
PWIN = WIN // 2          # 512 pairs per window
KDIM = 128               # contraction rows
NCAND = NWIN * 8         # 64 pair candidates per row
RESCORE = 24             # exact-rescored pair candidates per row

# gpsimd reads the pairwise max straight from PSUM; set False to route
# through an ACT copy to SBUF instead.
GPSIMD_FROM_PSUM = False

_COMPILED = {}
LAST_RUN = {}


def _build_nc():
    import concourse.bacc as bacc
    import concourse.mybir as mybir
    import concourse.tile as tile

    nc = bacc.Bacc(None)
    f32 = mybir.dt.float32
    f16 = mybir.dt.float16
    u32 = mybir.dt.uint32
    Act = mybir.ActivationFunctionType

    q_d = nc.declare_dram_parameter("q", [KDIM, QROWS], f16, isOutput=False)
    kv_d = nc.declare_dram_parameter("kv", [KDIM, M], f16, isOutput=False)
    cv_d = nc.declare_dram_parameter("cv", [NGROUPS, 128, NCAND], f32, isOutput=True)
    ci_d = nc.declare_dram_parameter("ci", [NGROUPS, 128, NCAND], u32, isOutput=True)

    with tile.TileContext(nc) as tc:
        with (
            tc.tile_pool(name="singles", bufs=1) as singles,
            tc.tile_pool(name="psum", bufs=4, space="PSUM") as psum,
            tc.tile_pool(name="wcopy", bufs=3) as wcopy,
            tc.tile_pool(name="mplane", bufs=2) as mpool,
            tc.tile_pool(name="cands", bufs=2) as cands,
        ):
            q_sb = singles.tile([KDIM, QROWS], f16)
            kv_sb = singles.tile([KDIM, M], f16)
            nc.gpsimd.dma_start(out=q_sb[:], in_=q_d[:])
            nc.gpsimd.dma_start(out=kv_sb[:], in_=kv_d[:])

            for g in range(NGROUPS):
                cv = cands.tile([128, NCAND], f32, tag="cv")
                ci = cands.tile([128, NCAND], u32, tag="ci")
                m = mpool.tile([128, NWIN * PWIN], f32, tag="m")
                lhsT = q_sb[:, g * 128:(g + 1) * 128]
                for w in range(NWIN):
                    pt = psum.tile([128, WIN], f32, tag="pt")
                    for hh in range(WIN // 512):
                        j0 = w * WIN + hh * 512
                        nc.tensor.matmul(
                            pt[:, hh * 512:(hh + 1) * 512], lhsT,
                            kv_sb[:, j0:j0 + 512], start=True, stop=True,
                        )
                    mw = m[:, w * PWIN:(w + 1) * PWIN]
                    wt = wcopy.tile([128, WIN], f32, tag="wt")
                    nc.scalar.activation(out=wt[:], in_=pt[:], func=Act.Copy)
                    nc.vector.tensor_max(mw, wt[:, 0:PWIN], wt[:, PWIN:WIN])
                    nc.vector.max(out=cv[:, w * 8:(w + 1) * 8], in_=mw)
                    nc.vector.max_index(
                        out=ci[:, w * 8:(w + 1) * 8],
                        in_max=cv[:, w * 8:(w + 1) * 8],
                        in_values=mw,
                    )
                nc.sync.dma_start(out=cv_d[g], in_=cv[:])
                nc.sync.dma_start(out=ci_d[g], in_=ci[:])
    if not nc.is_finalized():
        nc.finalize()
    return nc


def _split16(a):
    """fp16 hi/lo split of float64 array -> (hi, lo) fp16."""
    hi = a.astype(np.float16)
    lo = (a - hi.astype(np.float64)).astype(np.float16)
    return hi, lo


def _prep_inputs(x):
    """Per-core input dicts. x: (N, M, D) fp32."""
    x64 = x.astype(np.float64)
    qhi, qlo = _split16(x64)                     # (N, M, D)
    nrm = -0.5 * (x64 * x64).sum(-1)             # (N, M)
    nh, nl = _split16(nrm)

    in_maps = []
    for c in range(NCORES):
        b, h = c // 2, c % 2
        sl = slice(h * QROWS, (h + 1) * QROWS)
        q = np.zeros((KDIM, QROWS), np.float16)
        q[:D] = qhi[b, sl, :].T
        q[D:D + 62] = qlo[b, sl, :62].T
        q[126] = 1.0
        q[127] = 1.0
        kv = np.zeros((KDIM, M), np.float16)
        kv[:D] = qhi[b].T
        kv[D:D + 62] = qhi[b, :, :62].T
        kv[126] = nh[b]
        kv[127] = nl[b]
        in_maps.append({"q": q, "kv": kv})
    return in_maps


def kernel(x, k):
    x = np.asarray(x, dtype=np.float32)
    k = int(k)
    assert x.shape == (N, M, D) and k == K

    from concourse.bass_utils import run_bass_kernel_spmd

    if "nc" not in _COMPILED:
        _COMPILED["nc"] = _build_nc()
    nc = _COMPILED["nc"]

    in_maps = _prep_inputs(x)
    _r = run_bass_kernel_spmd(nc, in_maps, list(range(NCORES)))
    LAST_RUN["results"] = _r
    res = _r.results

    cv = np.empty((N, M, NCAND), np.float32)   # pair-max values
    ci = np.empty((N, M, NCAND), np.int64)     # pair idx within window (0..511)
    for c in range(NCORES):
        b, h = c // 2, c % 2
        sl = slice(h * QROWS, (h + 1) * QROWS)
        cv[b, sl] = res[c]["cv"].reshape(QROWS, NCAND)
        ci[b, sl] = res[c]["ci"].reshape(QROWS, NCAND)

    x64 = x.astype(np.float64)

    # ---- host merge: top-RESCORE pairs by value, expand to keys, rescore ----
    order = np.argsort(-cv, axis=-1, kind="stable")             # (N, M, 64)
    top = order[..., :RESCORE]
    pwin = top >> 3                                             # window id (0..7)
    ploc = np.take_along_axis(ci, top, axis=-1)                 # (N, M, 24)
    keyA = pwin * WIN + ploc                                    # first member
    kidx = np.empty((N, M, 2 * RESCORE), np.int64)              # 48 keys
    kidx[..., 0::2] = keyA
    kidx[..., 1::2] = keyA + PWIN                               # second member

    idx16 = np.empty((N, M, K), np.int64)
    d2_16 = np.empty((N, M, K), np.float64)
    for b in range(N):
        keys = x64[b][kidx[b]]                                  # (M, 48, 64)
        diff = keys - x64[b][:, None, :]
        d2 = np.einsum("mcd,mcd->mc", diff, diff)               # (M, 48)
        perm = np.lexsort((kidx[b], d2), axis=-1)[:, :K]
        idx16[b] = np.take_along_axis(kidx[b], perm, axis=-1)
        d2_16[b] = np.take_along_axis(d2, perm, axis=-1)

    # ---- suspect detection --------------------------------------------
    q2 = (x64 * x64).sum(-1)
    w16 = 0.5 * (q2[..., None] - d2_16[..., K - 1:K])           # exact w of 16th
    MARGIN = 0.1
    win8 = cv[..., 7::8].astype(np.float64)
    suspect = (win8 >= w16 - MARGIN).any(-1)
    v_sorted = np.take_along_axis(cv, order, axis=-1).astype(np.float64)
    suspect |= (v_sorted[..., RESCORE] >= w16[..., 0] - MARGIN)
    sv = np.sort(idx16, axis=-1)
    suspect |= (sv[..., 1:] == sv[..., :-1]).any(-1)
    # duplicate pair positions inside one window's top-8 (max_index collision)
    ps = np.sort(ci.reshape(N, M, NWIN, 8), axis=-1)
    suspect |= (ps[..., 1:] == ps[..., :-1]).any(-1).any(-1)

    nbad = int(suspect.sum())
    if nbad:
        for b in range(N):
            rows = np.nonzero(suspect[b])[0]
            if rows.size == 0:
                continue
            dif = x64[b][rows][:, None, :] - x64[b][None, :, :]
            d2r = np.einsum("rmd,rmd->rm", dif, dif)
            part = np.argpartition(d2r, K, axis=-1)[:, : K + 8]
            pd = np.take_along_axis(d2r, part, axis=-1)
            pperm = np.lexsort((part, pd), axis=-1)[:, :K]
            idx16[b, rows] = np.take_along_axis(part, pperm, axis=-1)

    offset = (np.arange(N, dtype=np.int64) * M)[:, None, None]
    src = (idx16 + offset).reshape(-1).astype(np.int32)
    dst = np.repeat(np.arange(N * M, dtype=np.int32), K)
    return src, dst


if __name__ == "__main__":
    rng = np.random.default_rng(0)
    xt = rng.standard_normal((N, M, D), dtype=np.float32)
    s, d = kernel(xt, 16)
    print(s[:32], d[:32])


# revision 12
# speedup vs baseline: 1.8356x; 1.0014x over previous
"""KNNGraph (k=16) Bass kernel for 8 NeuronCores.

Input: x (4, 8192, 64) fp32. Output: (src, dst) int32 edge arrays of the
16-NN graph per batch (self included), matching jax.lax.top_k(-d2) order.

Sharding: core c handles batch c//2, query rows (c%2)*4096 ... +4096,
against all 8192 keys of that batch (query-row sharding, keys replicated).

Device pipeline (per core), for each of 32 groups of 128 query rows,
with a pairwise-max tournament so the DVE only scans half the matrix:
  PE  : w = q.k - |k|^2/2 per 1024-key window (fp16 hi/lo split inputs,
        K=128 contraction, 2x N=512 matmuls into one PSUM tile)
  GPS : m_w = max(w[:, 0:512], w[:, 512:1024])  -- pair p = keys (p, p+512)
  DVE : per window: MAX8 + FIND_INDEX8 on the 512-wide m_w
        -> 64 (pair-value, pair-index) candidates per row
Host: expand top-24 pairs to 48 keys, exact fp64 rescore, order by
      (d2, idx); conservative suspect checks -> exact fallback.
"""

import numpy as np

N, M, D = 4, 8192, 64
K = 16
NCORES = 8
QROWS = M // 2           # query rows per core
NGROUPS = QROWS // 128   # 32
NWIN = 8                 # windows of 1024 keys
WIN = M // NWIN          # 1024
PWIN = WIN // 2          # 512 pairs per window
KDIM = 128               # contraction rows
NCAND = NWIN * 8         # 64 pair candidates per row
RESCORE = 24             # exact-rescored pair candidates per row

# gpsimd reads the pairwise max straight from PSUM; set False to route
# through an ACT copy to SBUF instead.
GPSIMD_FROM_PSUM = False

_COMPILED = {}
LAST_RUN = {}


def _build_nc():
    import concourse.bacc as bacc
    import concourse.mybir as mybir
    import concourse.tile as tile

    nc = bacc.Bacc(None)
    f32 = mybir.dt.float32
    f16 = mybir.dt.float16
    u32 = mybir.dt.uint32
    Act = mybir.ActivationFunctionType

    q_d = nc.declare_dram_parameter("q", [KDIM, QROWS], f16, isOutput=False)
    kv_d = nc.declare_dram_parameter("kv", [KDIM, M], f16, isOutput=False)
    cv_d = nc.declare_dram_parameter("cv", [NGROUPS, 128, NCAND], f32, isOutput=True)
    ci_d = nc.declare_dram_parameter("ci", [NGROUPS, 128, NCAND], u32, isOutput=True)

    with tile.TileContext(nc) as tc:
        with (
            tc.tile_pool(name="singles", bufs=1) as singles,
            tc.tile_pool(name="psum", bufs=4, space="PSUM") as psum,
            tc.tile_pool(name="wcopy", bufs=3) as wcopy,
            tc.tile_pool(name="mplane", bufs=2) as mpool,
            tc.tile_pool(name="cands", bufs=2) as cands,
        ):
            q_sb = singles.tile([KDIM, QROWS], f16)
            kv_sb = singles.tile([KDIM, M], f16)
            nc.gpsimd.dma_start(out=q_sb[:], in_=q_d[:])
            nc.gpsimd.dma_start(out=kv_sb[:], in_=kv_d[:])

            for g in range(NGROUPS):
                cv = cands.tile([128, NCAND], f32, tag="cv")
                ci = cands.tile([128, NCAND], u32, tag="ci")
                m = mpool.tile([128, NWIN * PWIN], f32, tag="m")
                lhsT = q_sb[:, g * 128:(g + 1) * 128]
                for w in range(NWIN):
                    pt = psum.tile([128, WIN], f32, tag="pt")
                    for hh in range(WIN // 512):
                        j0 = w * WIN + hh * 512
                        nc.tensor.matmul(
                            pt[:, hh * 512:(hh + 1) * 512], lhsT,
                            kv_sb[:, j0:j0 + 512], start=True, stop=True,
                        )
                    mw = m[:, w * PWIN:(w + 1) * PWIN]
                    wt = wcopy.tile([128, WIN], f32, tag="wt")
                    nc.scalar.activation(out=wt[:], in_=pt[:], func=Act.Copy)
                    nc.vector.tensor_max(mw, wt[:, 0:PWIN], wt[:, PWIN:WIN])
                # separate streams: each op's producer is >=7 ops earlier,
                # so the DVE never stalls on an adjacent dependency
                for w in range(NWIN):
                    mw = m[:, w * PWIN:(w + 1) * PWIN]
                    nc.vector.max(out=cv[:, w * 8:(w + 1) * 8], in_=mw)
                for w in range(NWIN):
                    mw = m[:, w * PWIN:(w + 1) * PWIN]
                    nc.vector.max_index(
                        out=ci[:, w * 8:(w + 1) * 8],
                        in_max=cv[:, w * 8:(w + 1) * 8],
                        in_values=mw,
                    )
                nc.sync.dma_start(out=cv_d[g], in_=cv[:])
                nc.sync.dma_start(out=ci_d[g], in_=ci[:])
    if not nc.is_finalized():
        nc.finalize()
    return nc


def _split16(a):
    """fp16 hi/lo split of float64 array -> (hi, lo) fp16."""
    hi = a.astype(np.float16)
    lo = (a - hi.astype(np.float64)).astype(np.float16)
    return hi, lo


def _prep_inputs(x):
    """Per-core input dicts. x: (N, M, D) fp32."""
    x64 = x.astype(np.float64)
    qhi, qlo = _split16(x64)                     # (N, M, D)
    nrm = -0.5 * (x64 * x64).sum(-1)             # (N, M)
    nh, nl = _split16(nrm)

    in_maps = []
    for c in range(NCORES):
        b, h = c // 2, c % 2
        sl = slice(h * QROWS, (h + 1) * QROWS)
        q = np.zeros((KDIM, QROWS), np.float16)
        q[:D] = qhi[b, sl, :].T
        q[D:D + 62] = qlo[b, sl, :62].T
        q[126] = 1.0
        q[127] = 1.0
        kv = np.zeros((KDIM, M), np.float16)
        kv[:D] = qhi[b].T
        kv[D:D + 62] = qhi[b, :, :62].T
        kv[126] = nh[b]
        kv[127] = nl[b]
        in_maps.append({"q": q, "kv": kv})
    return in_maps


def kernel(x, k):
    x = np.asarray(x, dtype=np.float32)
    k = int(k)
    assert x.shape == (N, M, D) and k == K

    from concourse.bass_utils import run_bass_kernel_spmd

    if "nc" not in _COMPILED:
        _COMPILED["nc"] = _build_nc()
    nc = _COMPILED["nc"]

    in_maps = _prep_inputs(x)
    _r = run_bass_kernel_spmd(nc, in_maps, list(range(NCORES)))
    LAST_RUN["results"] = _r
    res = _r.results

    cv = np.empty((N, M, NCAND), np.float32)   # pair-max values
    ci = np.empty((N, M, NCAND), np.int64)     # pair idx within window (0..511)
    for c in range(NCORES):
        b, h = c // 2, c % 2
        sl = slice(h * QROWS, (h + 1) * QROWS)
        cv[b, sl] = res[c]["cv"].reshape(QROWS, NCAND)
        ci[b, sl] = res[c]["ci"].reshape(QROWS, NCAND)

    x64 = x.astype(np.float64)

    # ---- host merge: top-RESCORE pairs by value, expand to keys, rescore ----
    order = np.argsort(-cv, axis=-1, kind="stable")             # (N, M, 64)
    top = order[..., :RESCORE]
    pwin = top >> 3                                             # window id (0..7)
    ploc = np.take_along_axis(ci, top, axis=-1)                 # (N, M, 24)
    keyA = pwin * WIN + ploc                                    # first member
    kidx = np.empty((N, M, 2 * RESCORE), np.int64)              # 48 keys
    kidx[..., 0::2] = keyA
    kidx[..., 1::2] = keyA + PWIN                               # second member

    x2_64 = (x64 * x64).sum(-1)                                 # (N, M) exact-ish
    idx16 = np.empty((N, M, K), np.int64)
    d2_16 = np.empty((N, M, K), np.float64)
    for b in range(N):
        keys = x[b][kidx[b]]                                    # (M, 48, 64) fp32
        dots = np.einsum("mcd,md->mc", keys, x[b], optimize=True)
        d2 = x2_64[b][:, None] + x2_64[b][kidx[b]] - 2.0 * dots.astype(np.float64)
        perm = np.lexsort((kidx[b], d2), axis=-1)[:, :K]
        idx16[b] = np.take_along_axis(kidx[b], perm, axis=-1)
        d2_16[b] = np.take_along_axis(d2, perm, axis=-1)

    # ---- suspect detection --------------------------------------------
    q2 = (x64 * x64).sum(-1)
    w16 = 0.5 * (q2[..., None] - d2_16[..., K - 1:K])           # exact w of 16th
    MARGIN = 0.1
    win8 = cv[..., 7::8].astype(np.float64)
    suspect = (win8 >= w16 - MARGIN).any(-1)
    v_sorted = np.take_along_axis(cv, order, axis=-1).astype(np.float64)
    suspect |= (v_sorted[..., RESCORE] >= w16[..., 0] - MARGIN)
    sv = np.sort(idx16, axis=-1)
    suspect |= (sv[..., 1:] == sv[..., :-1]).any(-1)
    # duplicate pair positions inside one window's top-8 (max_index collision)
    ps = np.sort(ci.reshape(N, M, NWIN, 8), axis=-1)
    suspect |= (ps[..., 1:] == ps[..., :-1]).any(-1).any(-1)

    nbad = int(suspect.sum())
    if nbad:
        for b in range(N):
            rows = np.nonzero(suspect[b])[0]
            if rows.size == 0:
                continue
            dif = x64[b][rows][:, None, :] - x64[b][None, :, :]
            d2r = np.einsum("rmd,rmd->rm", dif, dif)
            part = np.argpartition(d2r, K, axis=-1)[:, : K + 8]
            pd = np.take_along_axis(d2r, part, axis=-1)
            pperm = np.lexsort((part, pd), axis=-1)[:, :K]
            idx16[b, rows] = np.take_along_axis(part, pperm, axis=-1)

    offset = (np.arange(N, dtype=np.int64) * M)[:, None, None]
    src = (idx16 + offset).reshape(-1).astype(np.int32)
    dst = np.repeat(np.arange(N * M, dtype=np.int32), K)
    return src, dst


if __name__ == "__main__":
    rng = np.random.default_rng(0)
    xt = rng.standard_normal((N, M, D), dtype=np.float32)
    s, d = kernel(xt, 16)
    print(s[:32], d[:32])


# revision 17
# speedup vs baseline: 2.0309x; 1.1064x over previous
"""KNNGraph (k=16) Bass kernel for 8 NeuronCores.

Input: x (4, 8192, 64) fp32. Output: (src, dst) int32 edge arrays of the
16-NN graph per batch (self included), matching jax.lax.top_k(-d2) order.

Sharding: core c handles batch c//2, query rows (c%2)*4096 ... +4096,
against all 8192 keys of that batch (query-row sharding, keys replicated).

Device pipeline (per core), for each of 32 groups of 128 query rows,
with a pairwise-max tournament so the DVE only scans half the matrix:
  PE  : w = q.k - |k|^2/2 per 1024-key window (fp16 hi/lo split inputs,
        K=128 contraction, 2x N=512 matmuls into one PSUM tile)
  GPS : m_w = max(w[:, 0:512], w[:, 512:1024])  -- pair p = keys (p, p+512)
  DVE : per window: MAX8 + FIND_INDEX8 on the 512-wide m_w
        -> 64 (pair-value, pair-index) candidates per row
Host: expand top-24 pairs to 48 keys, exact fp64 rescore, order by
      (d2, idx); conservative suspect checks -> exact fallback.
"""

import numpy as np

N, M, D = 4, 8192, 64
K = 16
NCORES = 8
QROWS = M // 2           # query rows per core
NGROUPS = QROWS // 128   # 32
NWIN = 8                 # windows of 1024 keys
WIN = M // NWIN          # 1024
PWIN = WIN // 2          # 512 pairs per window (m1 plane)
GWIN = WIN // 4          # 256 4-key groups per window (m2 plane)
KDIM = 128               # contraction rows
NCAND = NWIN * 8         # 64 group candidates per row
RESCORE = 24             # exact-rescored group candidates per row (x4 keys)

# gpsimd reads the pairwise max straight from PSUM; set False to route
# through an ACT copy to SBUF instead.
GPSIMD_FROM_PSUM = False

_COMPILED = {}
LAST_RUN = {}


def _build_nc():
    import concourse.bacc as bacc
    import concourse.mybir as mybir
    import concourse.tile as tile

    nc = bacc.Bacc(None)
    f32 = mybir.dt.float32
    f16 = mybir.dt.float16
    u32 = mybir.dt.uint32
    Act = mybir.ActivationFunctionType

    q_d = nc.declare_dram_parameter("q", [KDIM, QROWS], f16, isOutput=False)
    kv_d = nc.declare_dram_parameter("kv", [KDIM, M], f16, isOutput=False)
    cv_d = nc.declare_dram_parameter("cv", [NGROUPS, 128, NCAND], f32, isOutput=True)
    ci_d = nc.declare_dram_parameter("ci", [NGROUPS, 128, NCAND], u32, isOutput=True)

    with tile.TileContext(nc) as tc:
        with (
            tc.tile_pool(name="singles", bufs=1) as singles,
            tc.tile_pool(name="psum", bufs=4, space="PSUM") as psum,
            tc.tile_pool(name="wcopy", bufs=3) as wcopy,
            tc.tile_pool(name="mplane", bufs=2) as mpool,
            tc.tile_pool(name="cands", bufs=2) as cands,
        ):
            q_sb = singles.tile([KDIM, QROWS], f16)
            kv_sb = singles.tile([KDIM, M], f16)
            nc.gpsimd.dma_start(out=q_sb[:], in_=q_d[:])
            nc.gpsimd.dma_start(out=kv_sb[:], in_=kv_d[:])

            for g in range(NGROUPS):
                cv = cands.tile([128, NCAND], f32, tag="cv")
                ci = cands.tile([128, NCAND], u32, tag="ci")
                m = mpool.tile([128, NWIN * PWIN], f32, tag="m")    # pair maxes
                m2 = mpool.tile([128, NWIN * GWIN], f32, tag="m2")  # 4-key maxes
                lhsT = q_sb[:, g * 128:(g + 1) * 128]
                for w in range(NWIN):
                    pt = psum.tile([128, WIN], f32, tag="pt")
                    for hh in range(WIN // 512):
                        j0 = w * WIN + hh * 512
                        nc.tensor.matmul(
                            pt[:, hh * 512:(hh + 1) * 512], lhsT,
                            kv_sb[:, j0:j0 + 512], start=True, stop=True,
                        )
                    mw = m[:, w * PWIN:(w + 1) * PWIN]
                    wt = wcopy.tile([128, WIN], f32, tag="wt")
                    nc.scalar.activation(out=wt[:], in_=pt[:], func=Act.Copy)
                    nc.vector.tensor_max(mw, wt[:, 0:PWIN], wt[:, PWIN:WIN])
                # level-2 combine: m2[w][p] = max(m1[w][p], m1[w][p+256])
                for w in range(NWIN):
                    mw = m[:, w * PWIN:(w + 1) * PWIN]
                    nc.vector.tensor_max(
                        m2[:, w * GWIN:(w + 1) * GWIN],
                        mw[:, 0:GWIN], mw[:, GWIN:PWIN],
                    )
                for w in range(NWIN):
                    m2w = m2[:, w * GWIN:(w + 1) * GWIN]
                    nc.vector.max(out=cv[:, w * 8:(w + 1) * 8], in_=m2w)
                for w in range(NWIN):
                    m2w = m2[:, w * GWIN:(w + 1) * GWIN]
                    nc.vector.max_index(
                        out=ci[:, w * 8:(w + 1) * 8],
                        in_max=cv[:, w * 8:(w + 1) * 8],
                        in_values=m2w,
                    )
                nc.sync.dma_start(out=cv_d[g], in_=cv[:])
                nc.sync.dma_start(out=ci_d[g], in_=ci[:])
    if not nc.is_finalized():
        nc.finalize()
    return nc


def _split16(a):
    """fp16 hi/lo split of float64 array -> (hi, lo) fp16."""
    hi = a.astype(np.float16)
    lo = (a - hi.astype(np.float64)).astype(np.float16)
    return hi, lo


def _prep_inputs(x):
    """Per-core input dicts. x: (N, M, D) fp32."""
    x64 = x.astype(np.float64)
    qhi, qlo = _split16(x64)                     # (N, M, D)
    nrm = -0.5 * (x64 * x64).sum(-1)             # (N, M)
    nh, nl = _split16(nrm)

    in_maps = []
    for c in range(NCORES):
        b, h = c // 2, c % 2
        sl = slice(h * QROWS, (h + 1) * QROWS)
        q = np.zeros((KDIM, QROWS), np.float16)
        q[:D] = qhi[b, sl, :].T
        q[D:D + 62] = qlo[b, sl, :62].T
        q[126] = 1.0
        q[127] = 1.0
        kv = np.zeros((KDIM, M), np.float16)
        kv[:D] = qhi[b].T
        kv[D:D + 62] = qhi[b, :, :62].T
        kv[126] = nh[b]
        kv[127] = nl[b]
        in_maps.append({"q": q, "kv": kv})
    return in_maps


def kernel(x, k):
    x = np.asarray(x, dtype=np.float32)
    k = int(k)
    assert x.shape == (N, M, D) and k == K

    from concourse.bass_utils import run_bass_kernel_spmd

    if "nc" not in _COMPILED:
        _COMPILED["nc"] = _build_nc()
    nc = _COMPILED["nc"]

    in_maps = _prep_inputs(x)
    _r = run_bass_kernel_spmd(nc, in_maps, list(range(NCORES)))
    LAST_RUN["results"] = _r
    res = _r.results

    cv = np.empty((N, M, NCAND), np.float32)   # pair-max values
    ci = np.empty((N, M, NCAND), np.int64)     # pair idx within window (0..511)
    for c in range(NCORES):
        b, h = c // 2, c % 2
        sl = slice(h * QROWS, (h + 1) * QROWS)
        cv[b, sl] = res[c]["cv"].reshape(QROWS, NCAND)
        ci[b, sl] = res[c]["ci"].reshape(QROWS, NCAND)

    x64 = x.astype(np.float64)

    # ---- host merge: top-RESCORE pairs by value, expand to keys, rescore ----
    order = np.argsort(-cv, axis=-1, kind="stable")             # (N, M, 64)
    top = order[..., :RESCORE]
    pwin = top >> 3                                             # window id (0..7)
    ploc = np.take_along_axis(ci, top, axis=-1)                 # (N, M, 24)
    keyA = pwin * WIN + ploc                                    # first member
    kidx = np.empty((N, M, 4 * RESCORE), np.int64)              # 96 keys
    kidx[..., 0::4] = keyA
    kidx[..., 1::4] = keyA + GWIN
    kidx[..., 2::4] = keyA + 2 * GWIN
    kidx[..., 3::4] = keyA + 3 * GWIN

    x2_64 = (x64 * x64).sum(-1)                                 # (N, M) exact-ish
    idx16 = np.empty((N, M, K), np.int64)
    d2_16 = np.empty((N, M, K), np.float64)
    for b in range(N):
        keys = x[b][kidx[b]]                                    # (M, 96, 64) fp32
        dots = np.einsum("mcd,md->mc", keys, x[b], optimize=True)
        d2 = x2_64[b][:, None] + x2_64[b][kidx[b]] - 2.0 * dots.astype(np.float64)
        perm = np.lexsort((kidx[b], d2), axis=-1)[:, :K]
        idx16[b] = np.take_along_axis(kidx[b], perm, axis=-1)
        d2_16[b] = np.take_along_axis(d2, perm, axis=-1)

    # ---- suspect detection --------------------------------------------
    q2 = (x64 * x64).sum(-1)
    w16 = 0.5 * (q2[..., None] - d2_16[..., K - 1:K])           # exact w of 16th
    MARGIN = 0.1
    win8 = cv[..., 7::8].astype(np.float64)
    suspect = (win8 >= w16 - MARGIN).any(-1)
    v_sorted = np.take_along_axis(cv, order, axis=-1).astype(np.float64)
    suspect |= (v_sorted[..., RESCORE] >= w16[..., 0] - MARGIN)
    sv = np.sort(idx16, axis=-1)
    suspect |= (sv[..., 1:] == sv[..., :-1]).any(-1)
    # duplicate pair positions inside one window's top-8 (max_index collision)
    ps = np.sort(ci.reshape(N, M, NWIN, 8), axis=-1)
    suspect |= (ps[..., 1:] == ps[..., :-1]).any(-1).any(-1)

    nbad = int(suspect.sum())
    if nbad:
        for b in range(N):
            rows = np.nonzero(suspect[b])[0]
            if rows.size == 0:
                continue
            dif = x64[b][rows][:, None, :] - x64[b][None, :, :]
            d2r = np.einsum("rmd,rmd->rm", dif, dif)
            part = np.argpartition(d2r, K, axis=-1)[:, : K + 8]
            pd = np.take_along_axis(d2r, part, axis=-1)
            pperm = np.lexsort((part, pd), axis=-1)[:, :K]
            idx16[b, rows] = np.take_along_axis(part, pperm, axis=-1)

    offset = (np.arange(N, dtype=np.int64) * M)[:, None, None]
    src = (idx16 + offset).reshape(-1).astype(np.int32)
    dst = np.repeat(np.arange(N * M, dtype=np.int32), K)
    return src, dst


if __name__ == "__main__":
    rng = np.random.default_rng(0)
    xt = rng.standard_normal((N, M, D), dtype=np.float32)
    s, d = kernel(xt, 16)
    print(s[:32], d[:32])


# revision 18
# speedup vs baseline: 2.0414x; 1.0051x over previous
"""KNNGraph (k=16) Bass kernel for 8 NeuronCores.

Input: x (4, 8192, 64) fp32. Output: (src, dst) int32 edge arrays of the
16-NN graph per batch (self included), matching jax.lax.top_k(-d2) order.

Sharding: core c handles batch c//2, query rows (c%2)*4096 ... +4096,
against all 8192 keys of that batch (query-row sharding, keys replicated).

Device pipeline (per core), for each of 32 groups of 128 query rows,
with a 2-level max tournament so the DVE only scans 1/4 of the matrix
with MAX8/FIND_INDEX8 (which run at 1 elem/cycle, vs TENSOR_TENSOR max
which streams 2 elems/cycle):
  PE  : w = q.k - |k|^2/2 per 1024-key window (fp16 hi/lo split inputs,
        K=128 contraction, 2x N=512 matmuls into one PSUM tile)
  ACT : copy w PSUM -> SBUF
  DVE : m1_w = max(w[:, 0:512], w[:, 512:1024])          (512 pairs)
        m2_w = max(m1_w[:, 0:256], m1_w[:, 256:512])     (256 4-key groups)
        per window: MAX8 + FIND_INDEX8 on the 256-wide m2_w
        -> 64 (group-value, group-index) candidates per row
        group p of window w covers keys w*1024 + {p, p+256, p+512, p+768}
Host: expand top-24 groups to 96 keys, exact rescore (fp32 dots, fp64
      combine), order by (d2, idx); conservative suspect checks ->
      exact fallback recompute for flagged rows.
"""

import numpy as np

N, M, D = 4, 8192, 64
K = 16
NCORES = 8
QROWS = M // 2           # query rows per core
NGROUPS = QROWS // 128   # 32
NWIN = 8                 # windows of 1024 keys
WIN = M // NWIN          # 1024
PWIN = WIN // 2          # 512 pairs per window (m1 plane)
GWIN = WIN // 4          # 256 4-key groups per window (m2 plane)
KDIM = 128               # contraction rows
NCAND = NWIN * 8         # 64 group candidates per row
RESCORE = 24             # exact-rescored group candidates per row (x4 keys)

# gpsimd reads the pairwise max straight from PSUM; set False to route
# through an ACT copy to SBUF instead.
GPSIMD_FROM_PSUM = False

_COMPILED = {}
LAST_RUN = {}


def _build_nc():
    import concourse.bacc as bacc
    import concourse.mybir as mybir
    import concourse.tile as tile

    nc = bacc.Bacc(None)
    f32 = mybir.dt.float32
    f16 = mybir.dt.float16
    u32 = mybir.dt.uint32
    Act = mybir.ActivationFunctionType

    q_d = nc.declare_dram_parameter("q", [KDIM, QROWS], f16, isOutput=False)
    kv_d = nc.declare_dram_parameter("kv", [KDIM, M], f16, isOutput=False)
    cv_d = nc.declare_dram_parameter("cv", [NGROUPS, 128, NCAND], f32, isOutput=True)
    ci_d = nc.declare_dram_parameter("ci", [NGROUPS, 128, NCAND], u32, isOutput=True)

    with tile.TileContext(nc) as tc:
        with (
            tc.tile_pool(name="singles", bufs=1) as singles,
            tc.tile_pool(name="psum", bufs=4, space="PSUM") as psum,
            tc.tile_pool(name="wcopy", bufs=3) as wcopy,
            tc.tile_pool(name="mplane", bufs=2) as mpool,
            tc.tile_pool(name="cands", bufs=2) as cands,
        ):
            q_sb = singles.tile([KDIM, QROWS], f16)
            kv_sb = singles.tile([KDIM, M], f16)
            nc.gpsimd.dma_start(out=q_sb[:], in_=q_d[:])
            nc.gpsimd.dma_start(out=kv_sb[:], in_=kv_d[:])

            for g in range(NGROUPS):
                cv = cands.tile([128, NCAND], f32, tag="cv")
                ci = cands.tile([128, NCAND], u32, tag="ci")
                m = mpool.tile([128, NWIN * PWIN], f32, tag="m")    # pair maxes
                m2 = mpool.tile([128, NWIN * GWIN], f32, tag="m2")  # 4-key maxes
                lhsT = q_sb[:, g * 128:(g + 1) * 128]
                for w in range(NWIN):
                    pt = psum.tile([128, WIN], f32, tag="pt")
                    for hh in range(WIN // 512):
                        j0 = w * WIN + hh * 512
                        nc.tensor.matmul(
                            pt[:, hh * 512:(hh + 1) * 512], lhsT,
                            kv_sb[:, j0:j0 + 512], start=True, stop=True,
                        )
                    mw = m[:, w * PWIN:(w + 1) * PWIN]
                    wt = wcopy.tile([128, WIN], f32, tag="wt")
                    nc.scalar.activation(out=wt[:], in_=pt[:], func=Act.Copy)
                    nc.vector.tensor_max(mw, wt[:, 0:PWIN], wt[:, PWIN:WIN])
                # level-2 combine: m2[w][p] = max(m1[w][p], m1[w][p+256])
                for w in range(NWIN):
                    mw = m[:, w * PWIN:(w + 1) * PWIN]
                    nc.vector.tensor_max(
                        m2[:, w * GWIN:(w + 1) * GWIN],
                        mw[:, 0:GWIN], mw[:, GWIN:PWIN],
                    )
                for w in range(NWIN):
                    m2w = m2[:, w * GWIN:(w + 1) * GWIN]
                    nc.vector.max(out=cv[:, w * 8:(w + 1) * 8], in_=m2w)
                for w in range(NWIN):
                    m2w = m2[:, w * GWIN:(w + 1) * GWIN]
                    nc.vector.max_index(
                        out=ci[:, w * 8:(w + 1) * 8],
                        in_max=cv[:, w * 8:(w + 1) * 8],
                        in_values=m2w,
                    )
                nc.sync.dma_start(out=cv_d[g], in_=cv[:])
                nc.sync.dma_start(out=ci_d[g], in_=ci[:])
    if not nc.is_finalized():
        nc.finalize()
    return nc


def _split16(a):
    """fp16 hi/lo split of float64 array -> (hi, lo) fp16."""
    hi = a.astype(np.float16)
    lo = (a - hi.astype(np.float64)).astype(np.float16)
    return hi, lo


def _prep_inputs(x):
    """Per-core input dicts. x: (N, M, D) fp32."""
    x64 = x.astype(np.float64)
    qhi, qlo = _split16(x64)                     # (N, M, D)
    nrm = -0.5 * (x64 * x64).sum(-1)             # (N, M)
    nh, nl = _split16(nrm)

    in_maps = []
    for c in range(NCORES):
        b, h = c // 2, c % 2
        sl = slice(h * QROWS, (h + 1) * QROWS)
        q = np.zeros((KDIM, QROWS), np.float16)
        q[:D] = qhi[b, sl, :].T
        q[D:D + 62] = qlo[b, sl, :62].T
        q[126] = 1.0
        q[127] = 1.0
        kv = np.zeros((KDIM, M), np.float16)
        kv[:D] = qhi[b].T
        kv[D:D + 62] = qhi[b, :, :62].T
        kv[126] = nh[b]
        kv[127] = nl[b]
        in_maps.append({"q": q, "kv": kv})
    return in_maps


def kernel(x, k):
    x = np.asarray(x, dtype=np.float32)
    k = int(k)
    assert x.shape == (N, M, D) and k == K

    from concourse.bass_utils import run_bass_kernel_spmd

    if "nc" not in _COMPILED:
        _COMPILED["nc"] = _build_nc()
    nc = _COMPILED["nc"]

    in_maps = _prep_inputs(x)
    _r = run_bass_kernel_spmd(nc, in_maps, list(range(NCORES)))
    LAST_RUN["results"] = _r
    res = _r.results

    cv = np.empty((N, M, NCAND), np.float32)   # pair-max values
    ci = np.empty((N, M, NCAND), np.int64)     # pair idx within window (0..511)
    for c in range(NCORES):
        b, h = c // 2, c % 2
        sl = slice(h * QROWS, (h + 1) * QROWS)
        cv[b, sl] = res[c]["cv"].reshape(QROWS, NCAND)
        ci[b, sl] = res[c]["ci"].reshape(QROWS, NCAND)

    x64 = x.astype(np.float64)

    # ---- host merge: top-RESCORE pairs by value, expand to keys, rescore ----
    order = np.argsort(-cv, axis=-1, kind="stable")             # (N, M, 64)
    top = order[..., :RESCORE]
    pwin = top >> 3                                             # window id (0..7)
    ploc = np.take_along_axis(ci, top, axis=-1)                 # (N, M, 24)
    keyA = pwin * WIN + ploc                                    # first member
    kidx = np.empty((N, M, 4 * RESCORE), np.int64)              # 96 keys
    kidx[..., 0::4] = keyA
    kidx[..., 1::4] = keyA + GWIN
    kidx[..., 2::4] = keyA + 2 * GWIN
    kidx[..., 3::4] = keyA + 3 * GWIN

    x2_64 = (x64 * x64).sum(-1)                                 # (N, M) exact-ish
    idx16 = np.empty((N, M, K), np.int64)
    d2_16 = np.empty((N, M, K), np.float64)
    for b in range(N):
        keys = x[b][kidx[b]]                                    # (M, 96, 64) fp32
        dots = np.einsum("mcd,md->mc", keys, x[b], optimize=True)
        d2 = x2_64[b][:, None] + x2_64[b][kidx[b]] - 2.0 * dots.astype(np.float64)
        perm = np.lexsort((kidx[b], d2), axis=-1)[:, :K]
        idx16[b] = np.take_along_axis(kidx[b], perm, axis=-1)
        d2_16[b] = np.take_along_axis(d2, perm, axis=-1)

    # ---- suspect detection --------------------------------------------
    q2 = (x64 * x64).sum(-1)
    w16 = 0.5 * (q2[..., None] - d2_16[..., K - 1:K])           # exact w of 16th
    MARGIN = 0.1
    win8 = cv[..., 7::8].astype(np.float64)
    suspect = (win8 >= w16 - MARGIN).any(-1)
    v_sorted = np.take_along_axis(cv, order, axis=-1).astype(np.float64)
    suspect |= (v_sorted[..., RESCORE] >= w16[..., 0] - MARGIN)
    sv = np.sort(idx16, axis=-1)
    suspect |= (sv[..., 1:] == sv[..., :-1]).any(-1)
    # duplicate pair positions inside one window's top-8 (max_index collision)
    ps = np.sort(ci.reshape(N, M, NWIN, 8), axis=-1)
    suspect |= (ps[..., 1:] == ps[..., :-1]).any(-1).any(-1)

    nbad = int(suspect.sum())
    if nbad:
        for b in range(N):
            rows = np.nonzero(suspect[b])[0]
            if rows.size == 0:
                continue
            dif = x64[b][rows][:, None, :] - x64[b][None, :, :]
            d2r = np.einsum("rmd,rmd->rm", dif, dif)
            part = np.argpartition(d2r, K, axis=-1)[:, : K + 8]
            pd = np.take_along_axis(d2r, part, axis=-1)
            pperm = np.lexsort((part, pd), axis=-1)[:, :K]
            idx16[b, rows] = np.take_along_axis(part, pperm, axis=-1)

    offset = (np.arange(N, dtype=np.int64) * M)[:, None, None]
    src = (idx16 + offset).reshape(-1).astype(np.int32)
    dst = np.repeat(np.arange(N * M, dtype=np.int32), K)
    return src, dst


if __name__ == "__main__":
    rng = np.random.default_rng(0)
    xt = rng.standard_normal((N, M, D), dtype=np.float32)
    s, d = kernel(xt, 16)
    print(s[:32], d[:32])


# revision 19
# speedup vs baseline: 2.1509x; 1.0537x over previous
"""KNNGraph (k=16) Bass kernel for 8 NeuronCores.

Input: x (4, 8192, 64) fp32. Output: (src, dst) int32 edge arrays of the
16-NN graph per batch (self included), matching jax.lax.top_k(-d2) order.

Sharding: core c handles batch c//2, query rows (c%2)*4096 ... +4096,
against all 8192 keys of that batch (query-row sharding, keys replicated).

Device pipeline (per core), for each of 32 groups of 128 query rows,
with a 2-level max tournament so the DVE only scans 1/4 of the matrix
with MAX8/FIND_INDEX8 (which run at 1 elem/cycle, vs TENSOR_TENSOR max
which streams 2 elems/cycle):
  PE  : w = q.k - |k|^2/2 per 1024-key window (fp16 hi/lo split inputs,
        K=128 contraction, 2x N=512 matmuls into one PSUM tile)
  ACT : copy w PSUM -> SBUF
  DVE : m1_w = max(w[:, 0:512], w[:, 512:1024])          (512 pairs)
        m2_w = max(m1_w[:, 0:256], m1_w[:, 256:512])     (256 4-key groups)
        per window: MAX8 + FIND_INDEX8 on the 256-wide m2_w
        -> 64 (group-value, group-index) candidates per row
        group p of window w covers keys w*1024 + {p, p+256, p+512, p+768}
Host: expand top-24 groups to 96 keys, exact rescore (fp32 dots, fp64
      combine), order by (d2, idx); conservative suspect checks ->
      exact fallback recompute for flagged rows.
"""

import numpy as np

N, M, D = 4, 8192, 64
K = 16
NCORES = 8
QROWS = M // 2           # query rows per core
NGROUPS = QROWS // 128   # 32
NWIN = 8                 # windows of 1024 keys
WIN = M // NWIN          # 1024
PWIN = WIN // 2          # 512 pairs per window (m1 plane)
GWIN = WIN // 4          # 256 4-key groups per window (m2 plane)
KDIM = 128               # contraction rows
NCAND = NWIN * 8         # 64 group candidates per row
RESCORE = 24             # exact-rescored group candidates per row (x4 keys)

# gpsimd reads the pairwise max straight from PSUM; set False to route
# through an ACT copy to SBUF instead.
GPSIMD_FROM_PSUM = False

_COMPILED = {}
LAST_RUN = {}


def _build_nc():
    import concourse.bacc as bacc
    import concourse.mybir as mybir
    import concourse.tile as tile

    nc = bacc.Bacc(None)
    f32 = mybir.dt.float32
    f16 = mybir.dt.float16
    u32 = mybir.dt.uint32
    Act = mybir.ActivationFunctionType

    q_d = nc.declare_dram_parameter("q", [KDIM, QROWS], f16, isOutput=False)
    kv_d = nc.declare_dram_parameter("kv", [KDIM, M], f16, isOutput=False)
    cv_d = nc.declare_dram_parameter("cv", [NGROUPS, 128, NCAND], f32, isOutput=True)
    ci_d = nc.declare_dram_parameter("ci", [NGROUPS, 128, NCAND], u32, isOutput=True)

    with tile.TileContext(nc) as tc:
        with (
            tc.tile_pool(name="singles", bufs=1) as singles,
            tc.tile_pool(name="psum", bufs=4, space="PSUM") as psum,
            tc.tile_pool(name="wcopy", bufs=3) as wcopy,
            tc.tile_pool(name="mplane", bufs=2) as mpool,
            tc.tile_pool(name="cands", bufs=2) as cands,
        ):
            q_sb = singles.tile([KDIM, QROWS], f16)
            kv_sb = singles.tile([KDIM, M], f16)
            nc.gpsimd.dma_start(out=q_sb[:], in_=q_d[:])
            nc.gpsimd.dma_start(out=kv_sb[:], in_=kv_d[:])

            for g in range(NGROUPS):
                cv = cands.tile([128, NCAND], f32, tag="cv")
                ci = cands.tile([128, NCAND], u32, tag="ci")
                m = mpool.tile([128, NWIN * PWIN], f32, tag="m")    # pair maxes
                m2 = mpool.tile([128, NWIN * GWIN], f32, tag="m2")  # 4-key maxes
                wt = wcopy.tile([128, M], f32, tag="wt")
                lhsT = q_sb[:, g * 128:(g + 1) * 128]
                for w in range(NWIN):
                    pt = psum.tile([128, WIN], f32, tag="pt")
                    for hh in range(WIN // 512):
                        j0 = w * WIN + hh * 512
                        nc.tensor.matmul(
                            pt[:, hh * 512:(hh + 1) * 512], lhsT,
                            kv_sb[:, j0:j0 + 512], start=True, stop=True,
                        )
                    nc.scalar.activation(
                        out=wt[:, w * WIN:(w + 1) * WIN], in_=pt[:], func=Act.Copy
                    )
                # level-1 combine, all 8 windows in one 3D-AP op:
                # m1[w][p] = max(w[w][p], w[w][p+512])
                wv = wt[:].rearrange("p (w u) -> p w u", w=NWIN, u=WIN)
                mv = m[:].rearrange("p (w u) -> p w u", w=NWIN, u=PWIN)
                nc.vector.tensor_max(mv, wv[:, :, 0:PWIN], wv[:, :, PWIN:WIN])
                # level-2 combine: m2[w][p] = max(m1[w][p], m1[w][p+256])
                m2v = m2[:].rearrange("p (w u) -> p w u", w=NWIN, u=GWIN)
                nc.vector.tensor_max(m2v, mv[:, :, 0:GWIN], mv[:, :, GWIN:PWIN])
                for w in range(NWIN):
                    m2w = m2[:, w * GWIN:(w + 1) * GWIN]
                    nc.vector.max(out=cv[:, w * 8:(w + 1) * 8], in_=m2w)
                for w in range(NWIN):
                    m2w = m2[:, w * GWIN:(w + 1) * GWIN]
                    nc.vector.max_index(
                        out=ci[:, w * 8:(w + 1) * 8],
                        in_max=cv[:, w * 8:(w + 1) * 8],
                        in_values=m2w,
                    )
                nc.sync.dma_start(out=cv_d[g], in_=cv[:])
                nc.sync.dma_start(out=ci_d[g], in_=ci[:])
    if not nc.is_finalized():
        nc.finalize()
    return nc


def _split16(a):
    """fp16 hi/lo split of float64 array -> (hi, lo) fp16."""
    hi = a.astype(np.float16)
    lo = (a - hi.astype(np.float64)).astype(np.float16)
    return hi, lo


def _prep_inputs(x):
    """Per-core input dicts. x: (N, M, D) fp32."""
    x64 = x.astype(np.float64)
    qhi, qlo = _split16(x64)                     # (N, M, D)
    nrm = -0.5 * (x64 * x64).sum(-1)             # (N, M)
    nh, nl = _split16(nrm)

    in_maps = []
    for c in range(NCORES):
        b, h = c // 2, c % 2
        sl = slice(h * QROWS, (h + 1) * QROWS)
        q = np.zeros((KDIM, QROWS), np.float16)
        q[:D] = qhi[b, sl, :].T
        q[D:D + 62] = qlo[b, sl, :62].T
        q[126] = 1.0
        q[127] = 1.0
        kv = np.zeros((KDIM, M), np.float16)
        kv[:D] = qhi[b].T
        kv[D:D + 62] = qhi[b, :, :62].T
        kv[126] = nh[b]
        kv[127] = nl[b]
        in_maps.append({"q": q, "kv": kv})
    return in_maps


def kernel(x, k):
    x = np.asarray(x, dtype=np.float32)
    k = int(k)
    assert x.shape == (N, M, D) and k == K

    from concourse.bass_utils import run_bass_kernel_spmd

    if "nc" not in _COMPILED:
        _COMPILED["nc"] = _build_nc()
    nc = _COMPILED["nc"]

    in_maps = _prep_inputs(x)
    _r = run_bass_kernel_spmd(nc, in_maps, list(range(NCORES)))
    LAST_RUN["results"] = _r
    res = _r.results

    cv = np.empty((N, M, NCAND), np.float32)   # pair-max values
    ci = np.empty((N, M, NCAND), np.int64)     # pair idx within window (0..511)
    for c in range(NCORES):
        b, h = c // 2, c % 2
        sl = slice(h * QROWS, (h + 1) * QROWS)
        cv[b, sl] = res[c]["cv"].reshape(QROWS, NCAND)
        ci[b, sl] = res[c]["ci"].reshape(QROWS, NCAND)

    x64 = x.astype(np.float64)

    # ---- host merge: top-RESCORE pairs by value, expand to keys, rescore ----
    order = np.argsort(-cv, axis=-1, kind="stable")             # (N, M, 64)
    top = order[..., :RESCORE]
    pwin = top >> 3                                             # window id (0..7)
    ploc = np.take_along_axis(ci, top, axis=-1)                 # (N, M, 24)
    keyA = pwin * WIN + ploc                                    # first member
    kidx = np.empty((N, M, 4 * RESCORE), np.int64)              # 96 keys
    kidx[..., 0::4] = keyA
    kidx[..., 1::4] = keyA + GWIN
    kidx[..., 2::4] = keyA + 2 * GWIN
    kidx[..., 3::4] = keyA + 3 * GWIN

    x2_64 = (x64 * x64).sum(-1)                                 # (N, M) exact-ish
    idx16 = np.empty((N, M, K), np.int64)
    d2_16 = np.empty((N, M, K), np.float64)
    for b in range(N):
        keys = x[b][kidx[b]]                                    # (M, 96, 64) fp32
        dots = np.einsum("mcd,md->mc", keys, x[b], optimize=True)
        d2 = x2_64[b][:, None] + x2_64[b][kidx[b]] - 2.0 * dots.astype(np.float64)
        perm = np.lexsort((kidx[b], d2), axis=-1)[:, :K]
        idx16[b] = np.take_along_axis(kidx[b], perm, axis=-1)
        d2_16[b] = np.take_along_axis(d2, perm, axis=-1)

    # ---- suspect detection --------------------------------------------
    q2 = (x64 * x64).sum(-1)
    w16 = 0.5 * (q2[..., None] - d2_16[..., K - 1:K])           # exact w of 16th
    MARGIN = 0.1
    win8 = cv[..., 7::8].astype(np.float64)
    suspect = (win8 >= w16 - MARGIN).any(-1)
    v_sorted = np.take_along_axis(cv, order, axis=-1).astype(np.float64)
    suspect |= (v_sorted[..., RESCORE] >= w16[..., 0] - MARGIN)
    sv = np.sort(idx16, axis=-1)
    suspect |= (sv[..., 1:] == sv[..., :-1]).any(-1)
    # duplicate pair positions inside one window's top-8 (max_index collision)
    ps = np.sort(ci.reshape(N, M, NWIN, 8), axis=-1)
    suspect |= (ps[..., 1:] == ps[..., :-1]).any(-1).any(-1)

    nbad = int(suspect.sum())
    if nbad:
        for b in range(N):
            rows = np.nonzero(suspect[b])[0]
            if rows.size == 0:
                continue
            dif = x64[b][rows][:, None, :] - x64[b][None, :, :]
            d2r = np.einsum("rmd,rmd->rm", dif, dif)
            part = np.argpartition(d2r, K, axis=-1)[:, : K + 8]
            pd = np.take_along_axis(d2r, part, axis=-1)
            pperm = np.lexsort((part, pd), axis=-1)[:, :K]
            idx16[b, rows] = np.take_along_axis(part, pperm, axis=-1)

    offset = (np.arange(N, dtype=np.int64) * M)[:, None, None]
    src = (idx16 + offset).reshape(-1).astype(np.int32)
    dst = np.repeat(np.arange(N * M, dtype=np.int32), K)
    return src, dst


if __name__ == "__main__":
    rng = np.random.default_rng(0)
    xt = rng.standard_normal((N, M, D), dtype=np.float32)
    s, d = kernel(xt, 16)
    print(s[:32], d[:32])


# revision 23
# speedup vs baseline: 2.3266x; 1.0817x over previous
"""KNNGraph (k=16) Bass kernel for 8 NeuronCores.

Input: x (4, 8192, 64) fp32. Output: (src, dst) int32 edge arrays of the
16-NN graph per batch (self included), matching jax.lax.top_k(-d2) order.

Sharding: core c handles batch c//2, query rows (c%2)*4096 ... +4096,
against all 8192 keys of that batch (query-row sharding, keys replicated).

Device pipeline (per core), for each of 32 groups of 128 query rows,
with a 2-level max tournament so the DVE only scans 1/4 of the matrix
with MAX8/FIND_INDEX8 (which run at 1 elem/cycle, vs TENSOR_TENSOR max
which streams 2 elems/cycle):
  PE  : w = q.k - |k|^2/2 per 1024-key window (fp16 hi/lo split inputs,
        K=128 contraction, 2x N=512 matmuls into one PSUM tile)
  ACT : copy w PSUM -> SBUF
  DVE : m1_w = max(w[:, 0:512], w[:, 512:1024])          (512 pairs)
        m2_w = max(m1_w[:, 0:256], m1_w[:, 256:512])     (256 4-key groups)
        per window: MAX8 + FIND_INDEX8 on the 256-wide m2_w
        -> 64 (group-value, group-index) candidates per row
        group p of window w covers keys w*1024 + {p, p+256, p+512, p+768}
Host: expand top-24 groups to 96 keys, exact rescore (fp32 dots, fp64
      combine), order by (d2, idx); conservative suspect checks ->
      exact fallback recompute for flagged rows.
"""

import numpy as np

N, M, D = 4, 8192, 64
K = 16
NCORES = 8
QROWS = M // 2           # query rows per core
NGROUPS = QROWS // 128   # 32
NWIN = 8                 # windows of 1024 keys
WIN = M // NWIN          # 1024
PWIN = WIN // 2          # 512 pairs per window (m1 plane)
GWIN = WIN // 4          # 256 4-key groups per window (m2 plane)
HWIN = WIN // 8          # 128 8-key groups per window (m3 plane)
KDIM = 128               # contraction rows
NCAND = NWIN * 8         # 64 group candidates per row
RESCORE = 24             # exact-rescored group candidates per row (x8 keys)

# gpsimd reads the pairwise max straight from PSUM; set False to route
# through an ACT copy to SBUF instead.
GPSIMD_FROM_PSUM = False

_COMPILED = {}
LAST_RUN = {}


def _build_nc():
    import concourse.bacc as bacc
    import concourse.mybir as mybir
    import concourse.tile as tile

    nc = bacc.Bacc(None)
    f32 = mybir.dt.float32
    f16 = mybir.dt.float16
    u32 = mybir.dt.uint32
    Act = mybir.ActivationFunctionType

    q_d = nc.declare_dram_parameter("q", [KDIM, QROWS], f16, isOutput=False)
    kv_d = nc.declare_dram_parameter("kv", [KDIM, M], f16, isOutput=False)
    cv_d = nc.declare_dram_parameter("cv", [NGROUPS, 128, NCAND], f32, isOutput=True)
    ci_d = nc.declare_dram_parameter("ci", [NGROUPS, 128, NCAND], u32, isOutput=True)

    with tile.TileContext(nc) as tc:
        with (
            tc.tile_pool(name="singles", bufs=1) as singles,
            tc.tile_pool(name="psum", bufs=4, space="PSUM") as psum,
            tc.tile_pool(name="wcopy", bufs=3) as wcopy,
            tc.tile_pool(name="mplane", bufs=2) as mpool,
            tc.tile_pool(name="cands", bufs=2) as cands,
        ):
            q_sb = singles.tile([KDIM, QROWS], f16)
            kv_sb = singles.tile([KDIM, M], f16)
            nc.gpsimd.dma_start(out=q_sb[:], in_=q_d[:])
            nc.gpsimd.dma_start(out=kv_sb[:], in_=kv_d[:])

            for g in range(NGROUPS):
                cv = cands.tile([128, NCAND], f32, tag="cv")
                ci = cands.tile([128, NCAND], u32, tag="ci")
                m = mpool.tile([128, NWIN * PWIN], f32, tag="m")    # pair maxes
                m2 = mpool.tile([128, NWIN * GWIN], f32, tag="m2")  # 4-key maxes
                wt = wcopy.tile([128, M], f32, tag="wt")
                lhsT = q_sb[:, g * 128:(g + 1) * 128]
                for w in range(NWIN):
                    pt = psum.tile([128, WIN], f32, tag="pt")
                    for hh in range(WIN // 512):
                        j0 = w * WIN + hh * 512
                        nc.tensor.matmul(
                            pt[:, hh * 512:(hh + 1) * 512], lhsT,
                            kv_sb[:, j0:j0 + 512], start=True, stop=True,
                        )
                    nc.scalar.activation(
                        out=wt[:, w * WIN:(w + 1) * WIN], in_=pt[:], func=Act.Copy
                    )
                # level-1 combine, all 8 windows in one 3D-AP op:
                # m1[w][p] = max(w[w][p], w[w][p+512])
                wv = wt[:].rearrange("p (w u) -> p w u", w=NWIN, u=WIN)
                mv = m[:].rearrange("p (w u) -> p w u", w=NWIN, u=PWIN)
                nc.vector.tensor_max(mv, wv[:, :, 0:PWIN], wv[:, :, PWIN:WIN])
                # level-2 combine: m2[w][p] = max(m1[w][p], m1[w][p+256])
                m2v = m2[:].rearrange("p (w u) -> p w u", w=NWIN, u=GWIN)
                nc.vector.tensor_max(m2v, mv[:, :, 0:GWIN], mv[:, :, GWIN:PWIN])
                # level-3 combine: m3[w][p] = max(m2[w][p], m2[w][p+128])
                m3 = mpool.tile([128, NWIN * HWIN], f32, tag="m3")
                m3v = m3[:].rearrange("p (w u) -> p w u", w=NWIN, u=HWIN)
                nc.vector.tensor_max(m3v, m2v[:, :, 0:HWIN], m2v[:, :, HWIN:GWIN])
                for w in range(NWIN):
                    m3w = m3[:, w * HWIN:(w + 1) * HWIN]
                    nc.vector.max(out=cv[:, w * 8:(w + 1) * 8], in_=m3w)
                for w in range(NWIN):
                    m3w = m3[:, w * HWIN:(w + 1) * HWIN]
                    nc.vector.max_index(
                        out=ci[:, w * 8:(w + 1) * 8],
                        in_max=cv[:, w * 8:(w + 1) * 8],
                        in_values=m3w,
                    )
                nc.sync.dma_start(out=cv_d[g], in_=cv[:])
                nc.sync.dma_start(out=ci_d[g], in_=ci[:])
    if not nc.is_finalized():
        nc.finalize()
    return nc


def _split16(a):
    """fp16 hi/lo split of float64 array -> (hi, lo) fp16."""
    hi = a.astype(np.float16)
    lo = (a - hi.astype(np.float64)).astype(np.float16)
    return hi, lo


def _prep_inputs(x):
    """Per-core input dicts. x: (N, M, D) fp32."""
    x64 = x.astype(np.float64)
    qhi, qlo = _split16(x64)                     # (N, M, D)
    nrm = -0.5 * (x64 * x64).sum(-1)             # (N, M)
    nh, nl = _split16(nrm)

    in_maps = []
    for c in range(NCORES):
        b, h = c // 2, c % 2
        sl = slice(h * QROWS, (h + 1) * QROWS)
        q = np.zeros((KDIM, QROWS), np.float16)
        q[:D] = qhi[b, sl, :].T
        q[D:D + 62] = qlo[b, sl, :62].T
        q[126] = 1.0
        q[127] = 1.0
        kv = np.zeros((KDIM, M), np.float16)
        kv[:D] = qhi[b].T
        kv[D:D + 62] = qhi[b, :, :62].T
        kv[126] = nh[b]
        kv[127] = nl[b]
        in_maps.append({"q": q, "kv": kv})
    return in_maps


def kernel(x, k):
    x = np.asarray(x, dtype=np.float32)
    k = int(k)
    assert x.shape == (N, M, D) and k == K

    from concourse.bass_utils import run_bass_kernel_spmd

    if "nc" not in _COMPILED:
        _COMPILED["nc"] = _build_nc()
    nc = _COMPILED["nc"]

    in_maps = _prep_inputs(x)
    _r = run_bass_kernel_spmd(nc, in_maps, list(range(NCORES)))
    LAST_RUN["results"] = _r
    res = _r.results

    cv = np.empty((N, M, NCAND), np.float32)   # pair-max values
    ci = np.empty((N, M, NCAND), np.int64)     # pair idx within window (0..511)
    for c in range(NCORES):
        b, h = c // 2, c % 2
        sl = slice(h * QROWS, (h + 1) * QROWS)
        cv[b, sl] = res[c]["cv"].reshape(QROWS, NCAND)
        ci[b, sl] = res[c]["ci"].reshape(QROWS, NCAND)

    x64 = x.astype(np.float64)

    # ---- host merge: top-RESCORE pairs by value, expand to keys, rescore ----
    order = np.argsort(-cv, axis=-1, kind="stable")             # (N, M, 64)
    top = order[..., :RESCORE]
    pwin = top >> 3                                             # window id (0..7)
    ploc = np.take_along_axis(ci, top, axis=-1)                 # (N, M, 24)
    keyA = pwin * WIN + ploc                                    # first member
    kidx = np.empty((N, M, 8 * RESCORE), np.int64)              # 192 keys
    for j in range(8):
        kidx[..., j::8] = keyA + j * HWIN

    x2_64 = (x64 * x64).sum(-1)                                 # (N, M) exact-ish
    idx16 = np.empty((N, M, K), np.int64)
    d2_16 = np.empty((N, M, K), np.float64)
    for b in range(N):
        keys = x[b][kidx[b]]                                    # (M, 192, 64) fp32
        dots = np.einsum("mcd,md->mc", keys, x[b], optimize=True)
        d2 = x2_64[b][:, None] + x2_64[b][kidx[b]] - 2.0 * dots.astype(np.float64)
        perm = np.lexsort((kidx[b], d2), axis=-1)[:, :K]
        idx16[b] = np.take_along_axis(kidx[b], perm, axis=-1)
        d2_16[b] = np.take_along_axis(d2, perm, axis=-1)

    # ---- suspect detection --------------------------------------------
    q2 = (x64 * x64).sum(-1)
    w16 = 0.5 * (q2[..., None] - d2_16[..., K - 1:K])           # exact w of 16th
    MARGIN = 0.1
    win8 = cv[..., 7::8].astype(np.float64)
    suspect = (win8 >= w16 - MARGIN).any(-1)
    v_sorted = np.take_along_axis(cv, order, axis=-1).astype(np.float64)
    suspect |= (v_sorted[..., RESCORE] >= w16[..., 0] - MARGIN)
    sv = np.sort(idx16, axis=-1)
    suspect |= (sv[..., 1:] == sv[..., :-1]).any(-1)
    # duplicate pair positions inside one window's top-8 (max_index collision)
    ps = np.sort(ci.reshape(N, M, NWIN, 8), axis=-1)
    suspect |= (ps[..., 1:] == ps[..., :-1]).any(-1).any(-1)

    nbad = int(suspect.sum())
    if nbad:
        for b in range(N):
            rows = np.nonzero(suspect[b])[0]
            if rows.size == 0:
                continue
            dif = x64[b][rows][:, None, :] - x64[b][None, :, :]
            d2r = np.einsum("rmd,rmd->rm", dif, dif)
            part = np.argpartition(d2r, K, axis=-1)[:, : K + 8]
            pd = np.take_along_axis(d2r, part, axis=-1)
            pperm = np.lexsort((part, pd), axis=-1)[:, :K]
            idx16[b, rows] = np.take_along_axis(part, pperm, axis=-1)

    offset = (np.arange(N, dtype=np.int64) * M)[:, None, None]
    src = (idx16 + offset).reshape(-1).astype(np.int32)
    dst = np.repeat(np.arange(N * M, dtype=np.int32), K)
    return src, dst


if __name__ == "__main__":
    rng = np.random.default_rng(0)
    xt = rng.standard_normal((N, M, D), dtype=np.float32)
    s, d = kernel(xt, 16)
    print(s[:32], d[:32])
